# revision 20
# baseline (speedup 1.0000x reference)
"""Cross-attention (ALiBi) Trainium2 kernel — banded (sparse) attention.

Sharding: 8 cores = 2 batches x 4 head-groups. SPMD => one program for all
cores, so ALiBi bands are per member SLOT (union over groups), not per head.
Group g (cores g and 4+g) owns heads [12+g, 8+g, 4+g, g]: two head pairs
with nested band radii R_MEMBER = [inf, 442, 110.5, 27.7]:
  slot0 = pair (12+g, 8+g): full band for member0, member1 skips j-tiles
          outside R=442 (attn-v matmuls only; scores ride the pair).
  slot1 = pair (4+g, g): only j-tiles within R=110.5 of the i-chunk, the
          i-range of each kept tile trimmed to the band (16-aligned), and
          member1 further skips tiles outside R=27.7.

Per-core layouts:
  qT, kT  [256 e', 2048 n]  as SBUF [128, 2, 2048] bf16  (slot s tile s,
          member h2 at partitions h2*64..)
  v       [2048 n, 260]     as SBUF [128, 16, 65*4] bf16 (head k cols 65k..,
          ones col at 65k+64 accumulates the softmax denominator)
  scoresT [j, i] per head; ALiBi applied multiplicatively via a precomputed
  Toeplitz strip exp(-slope|j-i|) [128, 3968] per head; exp(s/8) on ACT.
  Score matmuls for the two slot members run concurrently via PE row tiling
  (K=64 at partitions 0 and 64). Banded attn-v accumulation zero-inits its
  PSUM banks with a K=1 outer-product matmul so variable i-ranges accumulate
  with consistent has_written state. q/k biases are folded into the
  PSUM->SBUF copy as per-partition tensor_scalar adds (no bias matmuls).
DMA: weights/inputs emitted in need-order as contiguous per-k chunks (the
Sync queue issues in order with 8 DMA semaphore slots; the 4MB ALiBi strip
goes last, chunked per head).
Normalization: denominators (ones-column row 64 of the o PSUM) inverted with
reciprocal_approx_fast (~5x faster than the iterative DVE reciprocal; needs
a base-partition-0 staging copy — the uop misbehaves at partition offsets),
broadcast on GpSimd, multiplied into oT. Host sums the 4 partials per batch
and adds bo.
"""

import sys
import numpy as np
import ml_dtypes
from contextlib import ExitStack

if "/opt/trn_rl_repo" not in sys.path:
    sys.path.insert(0, "/opt/trn_rl_repo")

B, N, E, H, D = 2, 2048, 1024, 16, 64
HPC = 4            # heads per core
ES = HPC * D       # 256 e'-columns per core
NCORES = 8
KT = E // 128      # 8 contraction tiles for projections
NT = N // 128      # 16 n/j tiles
NC512 = N // 512   # 4 chunks of 512
USTRIP = 3968      # Toeplitz strip width: u = f - (128*jt - 512*ic) + 1920
# Band radius per member index (SPMD => union over the groups' heads):
# member m of group g is head 4*(3-m)+g, so member bands are
# R(h12..15)=full, R(h8..11)<=R(11), R(h4..7)<=R(7), R(h0..3)<=R(3)
R_MEMBER = [1e9, 442.0, 110.5, 27.7]

BF16 = ml_dtypes.bfloat16

_cache: dict = {}


def _core_heads(g: int) -> list[int]:
    """Head order on core group g: descending band so pair members nest."""
    return [12 + g, 8 + g, 4 + g, g]


def _slot_tiles(R: float):
    """Per ic: list of (jt, lo, hi): kept j-tiles and 16-aligned i-windows."""
    out = []
    for ic in range(NC512):
        i0 = 512 * ic
        tiles = []
        for jt in range(NT):
            j0 = 128 * jt
            if j0 + 128 + R <= i0 or j0 - R >= i0 + 512:
                continue
            lo = max(0.0, j0 - R - i0)
            hi = min(512.0, j0 + 128 + R - i0)
            lo = int(np.floor(lo / 16) * 16)
            hi = int(np.ceil(hi / 16) * 16)
            tiles.append((jt, lo, hi))
        out.append(tiles)
    return out


MEMBER_TILES = [_slot_tiles(r) for r in R_MEMBER]
# pair (slot) tile list = member0's list (superset of member1's)
SLOT_TILES = [MEMBER_TILES[0], MEMBER_TILES[2]]
# member1-of-slot kept jt sets, for skipping its attn-v matmuls
M1_KEPT = [
    [{jt for (jt, _, _) in MEMBER_TILES[1][ic]} for ic in range(NC512)],
    [{jt for (jt, _, _) in MEMBER_TILES[3][ic]} for ic in range(NC512)],
]


def _alibi_slopes():
    return np.array([2.0 ** (-8.0 * (h + 1) / H) for h in range(H)], dtype=np.float64)


def _estrips():
    """[4 groups][4, 128, 3968] bf16: strip[p, u] = exp(-slope * |p + 1920 - u|)."""
    if "estrips" in _cache:
        return _cache["estrips"]
    slopes = _alibi_slopes()
    au = np.abs(np.arange(128)[:, None] + 1920 - np.arange(USTRIP)[None, :]).astype(np.float64)
    groups = []
    for g in range(4):
        heads = []
        for h in _core_heads(g):
            heads.append(np.exp(-slopes[h] * au))
        groups.append(np.stack(heads).astype(BF16))
    _cache["estrips"] = groups
    return groups


def _build():
    import concourse.bass as bass
    import concourse.mybir as mybir
    import concourse.tile as tile
    from concourse import bacc

    fp32 = mybir.dt.float32
    bf16 = mybir.dt.bfloat16
    AF = mybir.ActivationFunctionType

    nc = bacc.Bacc("TRN2", target_bir_lowering=False, debug=False)

    qtt = nc.dram_tensor("qtt", [E, N], bf16, kind="ExternalInput").ap()
    kvt = nc.dram_tensor("kvt", [E, N], bf16, kind="ExternalInput").ap()
    wq = nc.dram_tensor("wq", [E, ES], bf16, kind="ExternalInput").ap()
    wk = nc.dram_tensor("wk", [E, ES], bf16, kind="ExternalInput").ap()
    wv = nc.dram_tensor("wv", [E, ES], bf16, kind="ExternalInput").ap()
    wo = nc.dram_tensor("wo", [ES, E], bf16, kind="ExternalInput").ap()
    bq = nc.dram_tensor("bq", [128, 2], fp32, kind="ExternalInput").ap()
    bk = nc.dram_tensor("bk", [128, 2], fp32, kind="ExternalInput").ap()
    bv = nc.dram_tensor("bv", [1, ES], bf16, kind="ExternalInput").ap()
    estrip = nc.dram_tensor("estrip", [HPC, 128, USTRIP], bf16, kind="ExternalInput").ap()
    out = nc.dram_tensor("out", [N, E], fp32, kind="ExternalOutput").ap()

    with tile.TileContext(nc) as tc, ExitStack() as ctx:
        consts = ctx.enter_context(tc.tile_pool(name="consts", bufs=1))
        big = ctx.enter_context(tc.tile_pool(name="big", bufs=1))
        acts = ctx.enter_context(tc.tile_pool(name="acts", bufs=1))
        ptpool = ctx.enter_context(tc.tile_pool(name="ptpool", bufs=6))
        small = ctx.enter_context(tc.tile_pool(name="small", bufs=3))
        outsb = ctx.enter_context(tc.tile_pool(name="outsb", bufs=3))
        mmps = ctx.enter_context(tc.tile_pool(name="mmps", bufs=2, space="PSUM"))
        sps = ctx.enter_context(tc.tile_pool(name="sps", bufs=2, space="PSUM"))
        ops = ctx.enter_context(tc.tile_pool(name="ops", bufs=1, space="PSUM"))

        # ---- constants / weights into SBUF ----
        # DMA emission order matters: the Sync queue issues in order with only
        # 8 DMA semaphore slots, so put what the first matmuls need first and
        # the big estrip last (chunked so no 4MB transfer clogs a sem slot).
        wq_sb = consts.tile([128, KT, ES], bf16)
        wk_sb = consts.tile([128, KT, ES], bf16)
        bq_sb = consts.tile([128, 2], fp32)
        nc.sync.dma_start(bq_sb[:], bq)
        bk_sb = consts.tile([128, 2], fp32)
        nc.sync.dma_start(bk_sb[:], bk)

        qtt_sb = big.tile([128, KT, N], bf16)
        kvt_sb = big.tile([128, KT, N], bf16)
        # interleave per-k chunks (contiguous 2D DMAs) so the projection
        # k-loop can start as soon as the first chunks land
        for k in range(KT):
            nc.sync.dma_start(wq_sb[:, k, :], wq[k * 128:(k + 1) * 128, :])
            nc.sync.dma_start(wk_sb[:, k, :], wk[k * 128:(k + 1) * 128, :])
            nc.sync.dma_start(kvt_sb[:, k, :], kvt[k * 128:(k + 1) * 128, :])
            nc.sync.dma_start(qtt_sb[:, k, :], qtt[k * 128:(k + 1) * 128, :])

        wv_sb = consts.tile([128, KT, ES], bf16)
        nc.sync.dma_start(wv_sb[:], wv.rearrange("(t p) m -> p t m", p=128))
        bv_sb = consts.tile([1, ES], bf16)
        nc.sync.dma_start(bv_sb[:], bv)
        wo_sb = consts.tile([128, 2, E], bf16)
        nc.sync.dma_start(wo_sb[:], wo.rearrange("(t p) e -> p t e", p=128))
        es_sb = consts.tile([128, HPC, USTRIP], bf16)
        for h in range(HPC):
            nc.sync.dma_start(es_sb[:, h, :], estrip[h].rearrange("p u -> p u"))
        ones_bf = consts.tile([1, 512], bf16)
        nc.vector.memset(ones_bf[:], 1.0)
        zcol = consts.tile([1, 65], bf16)
        nc.vector.memset(zcol[:], 0.0)

        qT_sb = acts.tile([128, 2, N], bf16)
        kT_sb = acts.tile([128, 2, N], bf16)
        v_sb = acts.tile([128, NT, 65 * HPC], bf16)
        oT_sb = acts.tile([128, 2, N], bf16)

        # ---- q/k projections: out [e'=128 tile t, n chunk c] ----
        for t in range(2):
            for c in range(NC512):
                for (w_sb, b_sb, dst, src) in (
                    (wq_sb, bq_sb, qT_sb, qtt_sb),
                    (wk_sb, bk_sb, kT_sb, kvt_sb),
                ):
                    ps = mmps.tile([128, 512], fp32)
                    for k in range(KT):
                        nc.tensor.matmul(
                            ps[:],
                            w_sb[:, k, t * 128:(t + 1) * 128],
                            src[:, k, c * 512:(c + 1) * 512],
                            start=(k == 0), stop=(k == KT - 1),
                        )
                    # bias folded into the PSUM->SBUF copy (per-partition AP)
                    nc.vector.tensor_scalar_add(
                        dst[:, t, c * 512:(c + 1) * 512], ps[:], b_sb[:, t:t + 1])

        # ---- v projection: out [n tile jt, e'] + ones cols ----
        nc.vector.memset(v_sb[:, :, :].rearrange("p t (h c) -> p t h c", c=65)[:, :, :, 64:65], 1.0)
        for jt in range(NT):
            ps = mmps.tile([128, ES], fp32)
            for k in range(KT):
                nc.tensor.matmul(
                    ps[:],
                    kvt_sb[:, k, jt * 128:(jt + 1) * 128],
                    wv_sb[:, k, :],
                    start=(k == 0), stop=False,
                )
            nc.tensor.matmul(
                ps[:], ones_bf[:, 0:128], bv_sb[:], start=False, stop=True,
            )
            nc.vector.tensor_copy(
                v_sb[:, jt, :].rearrange("p (h c) -> p h c", c=65)[:, :, 0:64],
                ps[:].rearrange("p (h c) -> p h c", c=64),
            )

        # ---- banded attention: per (i-chunk, slot) ----
        for ic in range(NC512):
            isl0 = ic * 512
            for slot in range(2):
                tiles = SLOT_TILES[slot][ic]
                o2 = ops.tile([65, 2, 512], fp32, name="o2", tag="o2")
                # per-member kept jt sets (member1 has a narrower band)
                kept = [
                    {jt for (jt, _, _) in tiles},
                    M1_KEPT[slot][ic],
                ]
                banded = any(lo != 0 or hi != 512 for (_, lo, hi) in tiles)
                firstjt = [min(k) for k in kept]
                lastjt = [max(k) for k in kept]
                if banded:
                    # zero-init both banks so variable-range attn-v matmuls
                    # can accumulate with consistent has_written state
                    for h2 in range(2):
                        nc.tensor.matmul(
                            o2[:, h2, :], zcol[:], ones_bf[:, 0:512],
                            start=True, stop=False, skip_group_check=True,
                        )

                def emit_ot(jt, lo, hi, pt2):
                    for h2 in range(2):
                        if jt not in kept[h2]:
                            continue
                        hd = 2 * slot + h2
                        nc.tensor.matmul(
                            o2[:, h2, lo:hi],
                            v_sb[:, jt, hd * 65:hd * 65 + 65],
                            pt2[:, h2, lo:hi],
                            start=(jt == firstjt[h2] and not banded),
                            stop=(jt == lastjt[h2]),
                            skip_group_check=True,
                        )

                prev = None
                for (jt, lo, hi) in tiles:
                    s2 = sps.tile([128, 2, 512], fp32, tag="s_ps", name="s2")
                    for h2 in range(2):
                        if jt not in kept[h2]:
                            continue
                        hp = h2 * 64
                        nc.tensor.matmul(
                            s2[:, h2, lo:hi],
                            kT_sb[hp:hp + 64, slot, jt * 128:(jt + 1) * 128],
                            qT_sb[hp:hp + 64, slot, isl0 + lo:isl0 + hi],
                            start=True, stop=True,
                        )
                    pt2 = ptpool.tile([128, 2, 512], bf16, tag="pt", name="pt2")
                    if jt in kept[1]:
                        psl, esl = slice(None), slice(2 * slot, 2 * slot + 2)
                    else:
                        psl, esl = slice(0, 1), slice(2 * slot, 2 * slot + 1)
                    nc.scalar.activation(pt2[:, psl, lo:hi], s2[:, psl, lo:hi], AF.Exp, scale=0.125)
                    u0 = 1920 - 128 * jt + 512 * ic
                    nc.vector.tensor_mul(
                        pt2[:, psl, lo:hi], pt2[:, psl, lo:hi],
                        es_sb[:, esl, u0 + lo:u0 + hi],
                    )
                    if prev is not None:
                        emit_ot(*prev)
                    prev = (jt, lo, hi, pt2)
                emit_ot(*prev)

                # stage numerators+denominators out of PSUM, then normalize
                o_un = small.tile([65, 2, 512], fp32, tag="o_un", name="o_un")
                nc.scalar.copy(o_un[:, 0, :], o2[:, 0, :])
                nc.vector.tensor_copy(o_un[:, 1, :], o2[:, 1, :])
                dn = small.tile([1, 2, 512], fp32, tag="dn", name="dn")
                nc.vector.tensor_copy(dn[:], o_un[64:65, :, :])
                rcp = small.tile([1, 2, 512], fp32, tag="rcp", name="rcp")
                # approx_fast requires base_partition 0 (HW uop bug at p!=0)
                nc.vector.reciprocal_approx_fast(rcp[:], dn[:])
                for h2 in range(2):
                    hp = h2 * 64
                    rb = small.tile([64, 512], fp32, name="rb", tag="rb")
                    nc.gpsimd.partition_broadcast(rb[:], rcp[:, h2, :])
                    nc.vector.tensor_mul(
                        oT_sb[hp:hp + 64, slot, isl0:isl0 + 512],
                        o_un[0:64, h2, :], rb[:])

            # ---- output projection for this i-chunk ----
            for nt in range(4 * ic, 4 * ic + 4):
                for ec in range(2):
                    ps = mmps.tile([128, 512], fp32)
                    for t in range(2):
                        nc.tensor.matmul(
                            ps[:],
                            oT_sb[:, t, nt * 128:(nt + 1) * 128],
                            wo_sb[:, t, ec * 512:(ec + 1) * 512],
                            start=(t == 0), stop=(t == 1),
                        )
                    o_sb = outsb.tile([128, 512], fp32)
                    nc.any.tensor_copy(o_sb[:], ps[:])
                    nc.sync.dma_start(out[nt * 128:(nt + 1) * 128, ec * 512:(ec + 1) * 512], o_sb[:])

    nc.compile()
    return nc


def _get_nc():
    if "nc" not in _cache:
        _cache["nc"] = _build()
    return _cache["nc"]


def _in_maps(query, kv, Wq, bq, Wkv, bkv, Wo, bo):
    strips = _estrips()
    qT = [np.ascontiguousarray(query[b].T).astype(BF16) for b in range(B)]
    kvT = [np.ascontiguousarray(kv[b].T).astype(BF16) for b in range(B)]
    maps = []
    for c in range(NCORES):
        b, g = c // 4, c % 4
        idx = np.concatenate([np.arange(64 * h, 64 * h + 64) for h in _core_heads(g)])
        maps.append({
            "qtt": qT[b],
            "kvt": kvT[b],
            "wq": np.ascontiguousarray(Wq[:, idx]).astype(BF16),
            "wk": np.ascontiguousarray(Wkv[:, :E][:, idx]).astype(BF16),
            "wv": np.ascontiguousarray(Wkv[:, E:][:, idx]).astype(BF16),
            "wo": np.ascontiguousarray(Wo[idx, :]).astype(BF16),
            "bq": np.ascontiguousarray(bq[idx].reshape(2, 128).T).astype(np.float32),
            "bk": np.ascontiguousarray(bkv[:E][idx].reshape(2, 128).T).astype(np.float32),
            "bv": np.ascontiguousarray(bkv[E:][idx]).reshape(1, ES).astype(BF16),
            "estrip": strips[g],
        })
    return maps


def kernel(query, kv, Wq, bq, Wkv, bkv, Wo, bo, _collect=None):
    from concourse import bass_utils

    query = np.asarray(query, dtype=np.float32)
    kv = np.asarray(kv, dtype=np.float32)
    nc = _get_nc()
    maps = _in_maps(query, kv, np.asarray(Wq), np.asarray(bq), np.asarray(Wkv),
                    np.asarray(bkv), np.asarray(Wo), np.asarray(bo))
    res = bass_utils.run_bass_kernel_spmd(
        nc, maps, core_ids=list(range(NCORES)),
        **(_collect or {}),
    )
    if _collect is not None:
        _cache["last_results"] = res
    outp = np.zeros((B, N, E), dtype=np.float32)
    for c in range(NCORES):
        outp[c // 4] += res.results[c]["out"]
    outp += np.asarray(bo, dtype=np.float32)
    return outp
